# revision 19
# baseline (speedup 1.0000x reference)
"""Grouped-scale dequant GEMM (AxCoreLinearFP16) on 8 Trainium2 NeuronCores.

y[b,s,o] = sum_i x[b,s,i] * (weight[o,i] * scales[o, i//128])

Strategy: data-parallel over the flattened (b*s) rows — each core gets a
[1024, 4096] x-shard and the full weight/scales (no collectives). Per core:
  - x^T resident in SBUF via two 4 MiB DMA transposes (contraction dim on
    partitions: xT[p, ko, m] = x[m, ko*128+p])
  - w^T o-panels (512 wide) DMA-transposed in two 2 MiB chunks each; the
    128-wide k-chunk is exactly one quant group, so dequant needs one
    scales row per (o-chunk, k-chunk) broadcast across partitions. The
    broadcast runs as a rank-1 PE matmul (ones^T x row -> PSUM) and one
    in-place DVE multiply — broadcast-shaped DMAs (step-0 partition APs)
    and extra DRAM round-trips measurably stall the DMA pipeline on HW.
  - scales^T itself is built once on-chip (PE transposes of the [4096, 32]
    scales) and bounced through DRAM so each o-chunk's rows can be
    reloaded contiguously onto partition 0 for the rank-1 matmuls.
  - PE matmul accumulates over the 32 k-chunks into PSUM [128, 512] fp32;
    PSUM is evicted with a casting ACT copy and DMA'd out.

Workarounds for this environment's toolchain:
  - walrus here accepts only ONE sync-wait per instruction: extra waits are
    peeled onto same-engine NoOps (_split_multiwait_insts)
  - InstPartitionBroadcast ("ISA wrong length") and --enable-ldw-opt=true
    do not codegen; both are avoided.

Self-contained: hardcodes shapes from the problem spec.
"""

import sys

for _p in ("/opt/trn_rl_repo",):
    if _p not in sys.path:
        sys.path.insert(0, _p)

from contextlib import ExitStack

import numpy as np

import concourse.bass as bass
import concourse.mybir as mybir
import concourse.tile as tile
import bass_rust
from concourse.masks import make_identity


FP16 = mybir.dt.float16
FP32 = mybir.dt.float32

P = 128
NCORES = 8
B, S, IN, OUT = 4, 2048, 4096, 4096
GROUP = 128
M = B * S // NCORES          # 1024 rows of x per core
KO = IN // P                 # 32 k-chunks == quant groups
OC = 512                     # o-chunk (matmul free dim)
NOC = OUT // OC              # 8
MT = M // P                  # 8 m-tiles

_RUNNER = None


def _split_multiwait_insts(nc):
    """This env's walrus CoreV3 codegen accepts only one sync-wait per
    instruction; Tile's tail drain can carry one per DMAHW sem lane.
    Peel extra waits onto same-engine NoOps inserted just before."""
    ctr = 0
    for f in nc.m.functions:
        for bb in f.blocks:
            new = []
            for inst in bb.instructions:
                si = inst.sync_info
                if si is not None and si.on_wait and len(si.on_wait) > 1:
                    waits = list(si.on_wait)
                    for w in waits[:-1]:
                        ctr += 1
                        new.append(bass_rust.InstNoOp(
                            name=f"I-waitsplit-{ctr}",
                            engine=inst.engine,
                            sync_info=bass_rust.SyncInfo(on_wait=[w], on_update=[]),
                        ))
                    inst.sync_info = bass_rust.SyncInfo(
                        on_wait=[waits[-1]], on_update=list(si.on_update or [])
                    )
                new.append(inst)
            bb.instructions = new
    return ctr


def _build(M=M, IN=IN, OUT=OUT, wdeq_bufs=None, split_waits=True,
           scp_split=True, xpose_scalar=False, y_gpsimd=False, wraw_bufs=2,
           x_pe_transpose=True):
    # NOTE: xpose_scalar=True (DMA-transpose issued from the ACT HWDGE ring)
    # CORRUPTS DATA in this environment's toolchain — transposes must stay
    # on the sync ring. HW-bisected 2026-08-08.
    KO = IN // P
    NOC = OUT // OC
    MT = M // P
    nc = bass.Bass()
    x = nc.declare_dram_parameter("x", [M, IN], FP16, isOutput=False)
    w = nc.declare_dram_parameter("w", [OUT, IN], FP16, isOutput=False)
    s = nc.declare_dram_parameter("s", [OUT, KO], FP16, isOutput=False)
    y = nc.declare_dram_parameter("y", [M, OUT], FP16, isOutput=True)

    with tile.TileContext(nc) as tc, ExitStack() as ctx:
        const = ctx.enter_context(tc.tile_pool(name="const", bufs=1))
        scps = ctx.enter_context(tc.tile_pool(name="scps", bufs=2, space="PSUM"))
        dramp = ctx.enter_context(tc.tile_pool(name="dramp", bufs=1, space="DRAM"))
        xTp = ctx.enter_context(tc.tile_pool(name="xTp", bufs=1))
        wraw = ctx.enter_context(tc.tile_pool(name="wraw", bufs=wraw_bufs))
        scp0 = ctx.enter_context(tc.tile_pool(name="scp0", bufs=2))
        psb_pool = ctx.enter_context(tc.tile_pool(name="psb", bufs=2, space="PSUM"))
        ystg = ctx.enter_context(tc.tile_pool(name="ystg", bufs=4))
        psum = ctx.enter_context(tc.tile_pool(name="psum", bufs=4, space="PSUM"))

        if x_pe_transpose:
            xnp = ctx.enter_context(tc.tile_pool(name="xnp", bufs=2))

        # w o-panels via contiguous row-chunked DMA transposes. The xbar
        # transpose path is descriptor-rate-bound (~100-160 GB/s effective)
        # and single-ring (ACT-ring transposes corrupt data here), so w gets
        # the sync ring to itself — x stays off the xbar entirely (below).
        WC = 256                      # w rows per transpose chunk (2 MiB)

        def emit_panel_dma(oc):
            wr3 = wraw.tile([P, KO, OC], FP16, tag="wraw", name="wr3")
            for j in range(OC // WC):
                eng = nc.scalar if (xpose_scalar and j % 2 == 1) else nc.sync
                rows = slice(oc * OC + j * WC, oc * OC + (j + 1) * WC)
                eng.dma_start_transpose(out=wr3[:, :, j * WC:(j + 1) * WC],
                                        in_=w[rows, :])
            return wr3

        # Get the first two w panels onto the (exclusive) xbar ring ASAP.
        wrs = {0: emit_panel_dma(0), 1: emit_panel_dma(1)}

        # scales^T: one DMA loads all of scales partition-split, then PE
        # transposes + ACT copies build scT [KO, OUT], bounced through DRAM
        # (in per-oc chunks so panel 0's reload isn't gated on the full
        # build) so per-oc row-blocks land contiguously on partition 0 for
        # the rank-1 broadcast matmuls.
        ident = const.tile([P, P], FP16)
        make_identity(nc, ident)
        snat = const.tile([P, OUT // P, KO], FP16)
        sv = s[:, :].rearrange("(oo p) g -> p oo g", p=P)
        nc.gpsimd.dma_start(out=snat[:], in_=sv)
        scT = const.tile([KO, OUT], FP16)
        sT_dram = dramp.tile([KO, OUT], FP16)
        KG = 8                        # PE transposes batched per PSUM bank
        for og in range(OUT // P // KG):
            pst8 = scps.tile([P, KG, P], FP16, tag="xps")
            for j in range(KG):
                o2 = og * KG + j
                nc.tensor.transpose(pst8[0:KO, j, :], snat[:, o2, :], ident[:])
            gsl = slice(og * KG * P, (og + 1) * KG * P)
            nc.scalar.copy(out=scT[:, gsl], in_=pst8[0:KO, :, :])
            nc.gpsimd.dma_start(out=sT_dram[:, gsl], in_=scT[:, gsl])

        ones = const.tile([P, P], FP16)
        nc.gpsimd.memset(ones[:], 1.0)

        # x^T resident: load x NATURAL on the gpsimd (SWDGE) ring and
        # transpose [128,128] tiles on the PE (fp16 passthrough) into xT,
        # batching 8 tiles per PSUM bank and evicting with one DVE copy.
        # This keeps x's 8 MiB off the saturated xbar ring AND gives the PE
        # dense work from t~5us, warming the HAM clock before the main GEMM.
        xT = xTp.tile([P, KO, M], FP16)
        if x_pe_transpose:
            for mt in range(M // P):
                xn = xnp.tile([P, IN], FP16, tag="xn", name="xn")
                nc.gpsimd.dma_start(out=xn[:], in_=x[mt * P:(mt + 1) * P, :])
                for kg in range(KO // KG):
                    pst8 = scps.tile([P, KG, P], FP16, tag="xps")
                    for j in range(KG):
                        ko = kg * KG + j
                        nc.tensor.transpose(pst8[:, j, :],
                                            xn[:, ko * P:(ko + 1) * P],
                                            ident[:])
                    nc.vector.tensor_copy(
                        out=xT[:, kg * KG:(kg + 1) * KG, mt * P:(mt + 1) * P],
                        in_=pst8[:])
        else:
            XC = 256                  # x rows per transpose chunk (2 MiB)
            for i in range(M // XC):
                eng = nc.scalar if (xpose_scalar and i % 2 == 1) else nc.sync
                eng.dma_start_transpose(out=xT[:, :, i * XC:(i + 1) * XC],
                                        in_=x[i * XC:(i + 1) * XC, :])

        # Dequant per o-panel: the scales row for each k-chunk is broadcast
        # across partitions by a rank-1 PE matmul (ones^T x row) into PSUM,
        # then one DVE multiply applies it in place. (A DMA with a step-0
        # partition source AP measurably poisons the DMA pipeline on HW, so
        # the broadcast runs on the PE instead.) The KO scales rows are
        # split over partitions {0,32} (legal K=1 row-group bases) so the
        # scp tile only costs KO/2 rows of per-partition SBUF.
        KQ = KO // 2 if scp_split else KO
        NQ = KO // KQ

        def emit_dequant(oc, wr3):
            osl = slice(oc * OC, (oc + 1) * OC)
            scp = scp0.tile([32 * (NQ - 1) + 1, KQ, OC], FP16, tag="scp",
                            name="scp")
            for q in range(NQ):
                nc.scalar.dma_start(out=scp[32 * q:32 * q + 1, :, :],
                                    in_=sT_dram[q * KQ:(q + 1) * KQ, osl])
            for ko in range(KO):
                q, kq = divmod(ko, KQ)
                psb = psb_pool.tile([P, OC], FP32, tag="psb", name="psb")
                nc.tensor.matmul(psb[:], ones[32 * q:32 * q + 1, :],
                                 scp[32 * q:32 * q + 1, kq, :],
                                 start=True, stop=True)
                nc.vector.tensor_mul(wr3[:, ko, :], wr3[:, ko, :], psb[:])

        def emit_compute(oc, wr3):
            osl = slice(oc * OC, (oc + 1) * OC)
            for m in range(MT):
                pt = psum.tile([P, OC], FP32, name="pt")
                for ko in range(KO):
                    nc.tensor.matmul(
                        pt[:],
                        xT[:, ko, m * P:(m + 1) * P],
                        wr3[:, ko, :],
                        start=(ko == 0),
                        stop=(ko == KO - 1),
                    )
                yt = ystg.tile([P, OC], FP16, name="yt")
                nc.scalar.copy(out=yt[:], in_=pt[:])
                yeng = nc.gpsimd if y_gpsimd else nc.scalar
                yeng.dma_start(out=y[m * P:(m + 1) * P, osl], in_=yt[:])

        # Software pipeline: DMA two panels ahead, dequant one panel ahead,
        # so panel-boundary stalls (and the HAM re-throttle they trigger)
        # disappear.
        emit_dequant(0, wrs[0])
        for oc in range(NOC):
            if oc + 2 < NOC:
                wrs[oc + 2] = emit_panel_dma(oc + 2)
            if oc + 1 < NOC:
                emit_dequant(oc + 1, wrs[oc + 1])
            emit_compute(oc, wrs.pop(oc))

    if split_waits:
        _split_multiwait_insts(nc)
    return nc


def _get_runner():
    """Compile once; return a reusable callable mapping per-core input maps
    to per-core output maps (modeled on bass2jax.run_bass_via_pjrt)."""
    global _RUNNER
    if _RUNNER is not None:
        return _RUNNER

    import jax
    from jax.experimental.shard_map import shard_map
    from jax.sharding import Mesh, PartitionSpec
    from concourse import bass2jax

    nc = _build()
    bass2jax.install_neuronx_cc_hook()

    partition_name = nc.partition_id_tensor.name if nc.partition_id_tensor else None
    in_names, out_names, out_avals, zero_shapes = [], [], [], []
    for alloc in nc.m.functions[0].allocations:
        if not isinstance(alloc, mybir.MemoryLocationSet):
            continue
        name = alloc.memorylocations[0].name
        if alloc.kind == "ExternalInput":
            if name != partition_name:
                in_names.append(name)
        elif alloc.kind == "ExternalOutput":
            shape = tuple(alloc.tensor_shape)
            dtype = mybir.dt.np(alloc.dtype)
            out_names.append(name)
            out_avals.append(jax.core.ShapedArray(shape, dtype))
            zero_shapes.append((shape, dtype))
    n_params = len(in_names)
    n_outs = len(out_names)
    all_names = in_names + out_names
    if partition_name is not None:
        all_names = all_names + [partition_name]
    donate = tuple(range(n_params, n_params + n_outs))

    def _make_body(reps):
        def _body(*args):
            ins = list(args[:n_params])
            outs = list(args[n_params:n_params + n_outs])
            for _ in range(reps):
                operands = ins + outs
                if partition_name is not None:
                    operands.append(bass2jax.partition_id_tensor())
                outs = list(bass2jax._bass_exec_p.bind(
                    *operands,
                    out_avals=tuple(out_avals),
                    in_names=tuple(all_names),
                    out_names=tuple(out_names),
                    lowering_input_output_aliases=(),
                    sim_require_finite=True,
                    sim_require_nnan=True,
                    nc=nc,
                ))
            return tuple(outs)
        return _body

    devices = jax.devices()[:NCORES]
    mesh = Mesh(np.asarray(devices), ("core",))

    def _make_exec(reps):
        return jax.jit(
            shard_map(
                _make_body(reps),
                mesh=mesh,
                in_specs=(PartitionSpec("core"),) * (n_params + n_outs),
                out_specs=(PartitionSpec("core"),) * n_outs,
                check_rep=False,
            ),
            donate_argnums=donate,
            keep_unused=True,
        )

    sharded = _make_exec(1)
    _exec_cache = {1: sharded}
    from jax.sharding import NamedSharding
    shard = NamedSharding(mesh, PartitionSpec("core"))

    class Runner:
        def __init__(self):
            self.in_names = in_names
            self.out_names = out_names

        def put_inputs(self, in_maps):
            """Concat per-core inputs and place them on the mesh."""
            import jax as _jax
            concat_in = [
                np.concatenate([np.asarray(m[name]) for m in in_maps], axis=0)
                for name in in_names
            ]
            return [_jax.device_put(a, shard) for a in concat_in]

        def fresh_outs(self):
            import jax as _jax
            return [
                _jax.device_put(np.zeros((NCORES * sh[0], *sh[1:]), dt), shard)
                for sh, dt in zero_shapes
            ]

        def exec_dev(self, dev_in, dev_outs, reps=1):
            """Device step(s). dev_outs is donated; returns new out arrays
            (same shape/sharding — reusable as the next call's dev_outs,
            since the kernel overwrites every output element). reps>1
            chains that many NEFF executions inside one dispatch."""
            if reps not in _exec_cache:
                _exec_cache[reps] = _make_exec(reps)
            return _exec_cache[reps](*dev_in, *dev_outs)

        def run(self, in_maps):
            dev_in = self.put_inputs(in_maps)
            out_arrs = self.exec_dev(dev_in, self.fresh_outs())
            return [
                {
                    name: np.asarray(out_arrs[i]).reshape(
                        NCORES, *out_avals[i].shape)[c]
                    for i, name in enumerate(out_names)
                }
                for c in range(NCORES)
            ]

    _RUNNER = Runner()
    return _RUNNER


def kernel(x, weight, scales):
    runner = _get_runner()
    xf = np.ascontiguousarray(np.asarray(x, dtype=np.float16).reshape(B * S, IN))
    w = np.ascontiguousarray(np.asarray(weight, dtype=np.float16))
    s = np.ascontiguousarray(np.asarray(scales, dtype=np.float16))
    in_maps = [
        {"x": xf[c * M:(c + 1) * M], "w": w, "s": s} for c in range(NCORES)
    ]
    outs = runner.run(in_maps)
    yf = np.concatenate([outs[c]["y"] for c in range(NCORES)], axis=0)
    return yf.reshape(B, S, OUT).astype(np.float16)



# revision 44
# speedup vs baseline: 1.1286x; 1.1286x over previous
"""Grouped-scale dequant GEMM (AxCoreLinearFP16) on 8 Trainium2 NeuronCores.

y[b,s,o] = sum_i x[b,s,i] * (weight[o,i] * scales[o, i//128])

Strategy: data-parallel over the flattened (b*s) rows — each core gets a
[1024, 4096] x-shard and the full weight/scales (no collectives). Per core:
  - w^T o-panels (512 wide) DMA-transposed via the xbar on the sync HWDGE
    ring ONLY (the xbar path is descriptor-rate-bound ~100-160 GB/s and
    ACT-ring transposes corrupt data here), 3 panels in flight; panel
    requests are issued at the END of each compute block so the xbar's tiny
    packets don't monopolize the 16 shared SDMA engines.
  - x is loaded NATURAL (full-row DMAs) and transposed on the PE with plain
    identity matmuls into a resident xT [128, 32, 1024] — this keeps x off
    the saturated xbar ring AND, unlike transpose-mode ops, counts as
    PE-busy for the HAM activity monitor, warming the 2.4 GHz clock from
    ~15us. PSUM->xT eviction runs on ACT (the DVE FIFO carries the dequant
    muls, which wait on panel arrivals — head-of-line hazard).
  - dequant: one scales row per (o-chunk, k-chunk) is broadcast across
    partitions by a rank-1 PE matmul (ones^T x row -> PSUM) + in-place DVE
    multiply; the scales rows are split over partitions {0,32} (row-group
    bases) to halve scp SBUF. scales^T is built once on-chip (PE
    transposes) and bounced through DRAM in per-panel chunks. The NEXT
    panel's dequant is staged into the back half of THIS panel's m-loop.
  - PE matmul accumulates over the 32 k-chunks into PSUM [128, 512] fp32;
    PSUM is evicted with a casting ACT copy and DMA'd out.
  - HW-measured: 2048 main MMs stream at the warm 216 ns back-to-back rate;
    any >=3.4us PE gap re-throttles HAM to 1.2 GHz for ~57us, so the whole
    pipeline is shaped to keep the PE FIFO non-blocking.

Workarounds for this environment's toolchain:
  - walrus here accepts only ONE sync-wait per instruction: extra waits are
    peeled onto same-engine NoOps (_split_multiwait_insts)
  - InstPartitionBroadcast ("ISA wrong length"), --enable-ldw-opt=true, and
    dma_start_transpose on the ACT ring (data corruption) are all avoided.

Self-contained: hardcodes shapes from the problem spec.
"""

import sys

for _p in ("/opt/trn_rl_repo",):
    if _p not in sys.path:
        sys.path.insert(0, _p)

from contextlib import ExitStack

import numpy as np

import concourse.bass as bass
import concourse.mybir as mybir
import concourse.tile as tile
import bass_rust
from concourse.masks import make_identity


FP16 = mybir.dt.float16
FP32 = mybir.dt.float32

P = 128
NCORES = 8
B, S, IN, OUT = 4, 2048, 4096, 4096
GROUP = 128
M = B * S // NCORES          # 1024 rows of x per core
KO = IN // P                 # 32 k-chunks == quant groups
OC = 512                     # o-chunk (matmul free dim)
NOC = OUT // OC              # 8
MT = M // P                  # 8 m-tiles

_RUNNER = None


def _split_multiwait_insts(nc):
    """This env's walrus CoreV3 codegen accepts only one sync-wait per
    instruction; Tile's tail drain can carry one per DMAHW sem lane.
    Peel extra waits onto same-engine NoOps inserted just before."""
    ctr = 0
    for f in nc.m.functions:
        for bb in f.blocks:
            new = []
            for inst in bb.instructions:
                si = inst.sync_info
                if si is not None and si.on_wait and len(si.on_wait) > 1:
                    waits = list(si.on_wait)
                    for w in waits[:-1]:
                        ctr += 1
                        new.append(bass_rust.InstNoOp(
                            name=f"I-waitsplit-{ctr}",
                            engine=inst.engine,
                            sync_info=bass_rust.SyncInfo(on_wait=[w], on_update=[]),
                        ))
                    inst.sync_info = bass_rust.SyncInfo(
                        on_wait=[waits[-1]], on_update=list(si.on_update or [])
                    )
                new.append(inst)
            bb.instructions = new
    return ctr


def _build(M=M, IN=IN, OUT=OUT, wdeq_bufs=None, split_waits=True,
           scp_split=True, xpose_scalar=False, y_gpsimd=False, wraw_bufs=3,
           x_pe_transpose=True):
    # NOTE: xpose_scalar=True (DMA-transpose issued from the ACT HWDGE ring)
    # CORRUPTS DATA in this environment's toolchain — transposes must stay
    # on the sync ring. HW-bisected 2026-08-08.
    KO = IN // P
    NOC = OUT // OC
    MT = M // P
    nc = bass.Bass()
    x = nc.declare_dram_parameter("x", [M, IN], FP16, isOutput=False)
    w = nc.declare_dram_parameter("w", [OUT, IN], FP16, isOutput=False)
    s = nc.declare_dram_parameter("s", [OUT, KO], FP16, isOutput=False)
    y = nc.declare_dram_parameter("y", [M, OUT], FP16, isOutput=True)

    with tile.TileContext(nc) as tc, ExitStack() as ctx:
        const = ctx.enter_context(tc.tile_pool(name="const", bufs=1))
        scps = ctx.enter_context(tc.tile_pool(name="scps", bufs=2, space="PSUM"))
        dramp = ctx.enter_context(tc.tile_pool(name="dramp", bufs=1, space="DRAM"))
        xTp = ctx.enter_context(tc.tile_pool(name="xTp", bufs=1))
        wraw = ctx.enter_context(tc.tile_pool(name="wraw", bufs=wraw_bufs))
        scp0 = ctx.enter_context(tc.tile_pool(name="scp0", bufs=1))
        psb_pool = ctx.enter_context(tc.tile_pool(name="psb", bufs=2, space="PSUM"))
        ystg = ctx.enter_context(tc.tile_pool(name="ystg", bufs=4))
        psum = ctx.enter_context(tc.tile_pool(name="psum", bufs=4, space="PSUM"))

        if x_pe_transpose:
            xnp = ctx.enter_context(tc.tile_pool(name="xnp", bufs=2))

        # w o-panels via contiguous row-chunked DMA transposes. The xbar
        # transpose path is descriptor-rate-bound (~100-160 GB/s effective)
        # and single-ring (ACT-ring transposes corrupt data here), so w gets
        # the sync ring to itself — x stays off the xbar entirely (below).
        WC = 256                      # w rows per transpose chunk (2 MiB)

        def emit_panel_dma(oc):
            wr3 = wraw.tile([P, KO, OC], FP16, tag="wraw", name="wr3")
            for j in range(OC // WC):
                eng = nc.scalar if (xpose_scalar and j % 2 == 1) else nc.sync
                rows = slice(oc * OC + j * WC, oc * OC + (j + 1) * WC)
                eng.dma_start_transpose(out=wr3[:, :, j * WC:(j + 1) * WC],
                                        in_=w[rows, :])
            return wr3

        # scales^T: one DMA loads all of scales partition-split, then PE
        # transposes + ACT copies build scT [KO, OUT], bounced through DRAM
        # (in per-oc chunks so panel 0's reload isn't gated on the full
        # build) so per-oc row-blocks land contiguously on partition 0 for
        # the rank-1 broadcast matmuls.
        ident = const.tile([P, P], FP16)
        make_identity(nc, ident)
        snat = const.tile([P, OUT // P, KO], FP16)
        sv = s[:, :].rearrange("(oo p) g -> p oo g", p=P)
        nc.sync.dma_start(out=snat[:], in_=sv)
        # The xbar's tiny packets monopolize the shared SDMA engines, so the
        # first two xn loads go on the sync ring AHEAD of panel 0, and later
        # panels are requested only at the END of each compute block —
        # otherwise the x stream starves (HW-measured).
        wrs = {}

        # x^T resident: load x NATURAL on the gpsimd (SWDGE) ring and
        # transpose [128,128] tiles on the PE (fp16 passthrough) into xT,
        # batching 8 tiles per PSUM bank and evicting with one DVE copy.
        # This keeps x's 8 MiB off the saturated xbar ring AND gives the PE
        # dense work from t~5us, warming the HAM clock before the main GEMM.
        xT = xTp.tile([P, KO, M], FP16)
        XG = 4                        # x transposes batched per PSUM bank

        def emit_x_mtile(mt, eng=None):
            """Load one m-tile of x naturally (full 8 KiB rows) and
            transpose it into xT via PLAIN identity matmuls (fp32 PSUM):
            unlike transpose-mode ops these count as PE-busy for the HAM
            activity monitor, so the startup phase warms the clock gate."""
            xn = xnp.tile([P, IN], FP16, tag="xn", name="xn")
            (eng or nc.scalar).dma_start(out=xn[:],
                                         in_=x[mt * P:(mt + 1) * P, :])
            for kg in range(KO // XG):
                pst4 = scps.tile([P, XG, P], FP32, tag="xps")
                for j in range(XG):
                    ko = kg * XG + j
                    nc.tensor.matmul(pst4[:, j, :],
                                     xn[:, ko * P:(ko + 1) * P],
                                     ident[:], start=True, stop=True)
                # ACT (not DVE) eviction: the DVE FIFO carries the dequant
                # muls, which wait on panel arrivals — x copies must not
                # queue behind them (head-of-line stall, HW-measured).
                nc.scalar.copy(
                    out=xT[:, kg * XG:(kg + 1) * XG, mt * P:(mt + 1) * P],
                    in_=pst4[:])

        if x_pe_transpose:
            emit_x_mtile(0, nc.sync)
            emit_x_mtile(1, nc.sync)
            wrs[0] = emit_panel_dma(0)
        else:
            wrs[0] = emit_panel_dma(0)
        if not x_pe_transpose:
            XC = 256                  # x rows per transpose chunk (2 MiB)
            for i in range(M // XC):
                eng = nc.scalar if (xpose_scalar and i % 2 == 1) else nc.sync
                eng.dma_start_transpose(out=xT[:, :, i * XC:(i + 1) * XC],
                                        in_=x[i * XC:(i + 1) * XC, :])

        # scale transposes + scT -> DRAM bounce (PE + ACT), after the
        # x stream so the scalar-ring xn loads fire first.
        scT = const.tile([KO, OUT], FP16)
        sT_dram = dramp.tile([KO, OUT], FP16)
        SG = 8                        # scale transposes batched per PSUM bank
        for og in range(OUT // P // SG):
            pst8 = scps.tile([P, SG, P], FP16, tag="xps")
            for j in range(SG):
                o2 = og * SG + j
                nc.tensor.transpose(pst8[0:KO, j, :], snat[:, o2, :], ident[:])
            gsl = slice(og * SG * P, (og + 1) * SG * P)
            nc.scalar.copy(out=scT[:, gsl], in_=pst8[0:KO, :, :])
            nc.scalar.dma_start(out=sT_dram[:, gsl], in_=scT[:, gsl])

        ones = const.tile([P, P], FP16)
        nc.gpsimd.memset(ones[:], 1.0)

        # Dequant per o-panel: the scales row for each k-chunk is broadcast
        # across partitions by a rank-1 PE matmul (ones^T x row) into PSUM,
        # then one DVE multiply applies it in place. (A DMA with a step-0
        # partition source AP measurably poisons the DMA pipeline on HW, so
        # the broadcast runs on the PE instead.) The KO scales rows are
        # split over partitions {0,32} (legal K=1 row-group bases) so the
        # scp tile only costs KO/2 rows of per-partition SBUF.
        KQ = KO // 2 if scp_split else KO
        NQ = KO // KQ

        def emit_scp_load(oc):
            osl = slice(oc * OC, (oc + 1) * OC)
            scp = scp0.tile([32 * (NQ - 1) + 1, KQ, OC], FP16, tag="scp",
                            name="scp")
            for q in range(NQ):
                nc.scalar.dma_start(out=scp[32 * q:32 * q + 1, :, :],
                                    in_=sT_dram[q * KQ:(q + 1) * KQ, osl])
            return scp

        def emit_dequant_chunk(scp, wr3, kos):
            for ko in kos:
                q, kq = divmod(ko, KQ)
                psb = psb_pool.tile([P, OC], FP32, tag="psb", name="psb")
                nc.tensor.matmul(psb[:], ones[32 * q:32 * q + 1, :],
                                 scp[32 * q:32 * q + 1, kq, :],
                                 start=True, stop=True)
                nc.vector.tensor_mul(wr3[:, ko, :], wr3[:, ko, :], psb[:])

        def emit_dequant(oc, wr3):
            emit_dequant_chunk(emit_scp_load(oc), wr3, range(KO))

        def emit_compute(oc, wr3, mtile_hook=None):
            osl = slice(oc * OC, (oc + 1) * OC)
            for m in range(MT):
                if mtile_hook is not None:
                    mtile_hook(m)
                pt = psum.tile([P, OC], FP32, name="pt")
                for ko in range(KO):
                    nc.tensor.matmul(
                        pt[:],
                        xT[:, ko, m * P:(m + 1) * P],
                        wr3[:, ko, :],
                        start=(ko == 0),
                        stop=(ko == KO - 1),
                    )
                yt = ystg.tile([P, OC], FP16, name="yt")
                nc.scalar.copy(out=yt[:], in_=pt[:])
                yeng = nc.gpsimd if y_gpsimd else nc.scalar
                yeng.dma_start(out=y[m * P:(m + 1) * P, osl], in_=yt[:])

        # Software pipeline: w DMA runs two panels ahead; the NEXT panel's
        # dequant (scp load + rank-1 broadcasts + DVE muls) is staged into
        # the back half of THIS panel's m-tile loop, so the PE FIFO never
        # parks on a broadcast whose panel hasn't landed yet. During panel
        # 0, the remaining x m-tile transposes fill the front half.
        emit_dequant(0, wrs[0])
        wrs[1] = emit_panel_dma(1)
        DSTAGES = 4                   # dequant spread over last 4 m-tiles
        DKO = KO // DSTAGES
        scp_next = [None]

        def make_hook(oc):
            def hook(m):
                if x_pe_transpose and oc == 0 and m < MT - 2:
                    emit_x_mtile(m + 2)
                if oc + 1 < NOC:
                    if m == MT - DSTAGES - 1:
                        scp_next[0] = emit_scp_load(oc + 1)
                    elif m >= MT - DSTAGES:
                        g = m - (MT - DSTAGES)
                        emit_dequant_chunk(scp_next[0], wrs[oc + 1],
                                           range(g * DKO, (g + 1) * DKO))
                if m == 2 and oc + 2 < NOC:
                    wrs[oc + 2] = emit_panel_dma(oc + 2)
            return hook

        for oc in range(NOC):
            emit_compute(oc, wrs[oc], mtile_hook=make_hook(oc))
            del wrs[oc]

    if split_waits:
        _split_multiwait_insts(nc)
    return nc


def _get_runner():
    """Compile once; return a reusable callable mapping per-core input maps
    to per-core output maps (modeled on bass2jax.run_bass_via_pjrt)."""
    global _RUNNER
    if _RUNNER is not None:
        return _RUNNER

    import jax
    from jax.experimental.shard_map import shard_map
    from jax.sharding import Mesh, PartitionSpec
    from concourse import bass2jax

    nc = _build()
    bass2jax.install_neuronx_cc_hook()

    partition_name = nc.partition_id_tensor.name if nc.partition_id_tensor else None
    in_names, out_names, out_avals, zero_shapes = [], [], [], []
    for alloc in nc.m.functions[0].allocations:
        if not isinstance(alloc, mybir.MemoryLocationSet):
            continue
        name = alloc.memorylocations[0].name
        if alloc.kind == "ExternalInput":
            if name != partition_name:
                in_names.append(name)
        elif alloc.kind == "ExternalOutput":
            shape = tuple(alloc.tensor_shape)
            dtype = mybir.dt.np(alloc.dtype)
            out_names.append(name)
            out_avals.append(jax.core.ShapedArray(shape, dtype))
            zero_shapes.append((shape, dtype))
    n_params = len(in_names)
    n_outs = len(out_names)
    all_names = in_names + out_names
    if partition_name is not None:
        all_names = all_names + [partition_name]
    donate = tuple(range(n_params, n_params + n_outs))

    def _make_body(reps):
        def _body(*args):
            ins = list(args[:n_params])
            outs = list(args[n_params:n_params + n_outs])
            for _ in range(reps):
                operands = ins + outs
                if partition_name is not None:
                    operands.append(bass2jax.partition_id_tensor())
                outs = list(bass2jax._bass_exec_p.bind(
                    *operands,
                    out_avals=tuple(out_avals),
                    in_names=tuple(all_names),
                    out_names=tuple(out_names),
                    lowering_input_output_aliases=(),
                    sim_require_finite=True,
                    sim_require_nnan=True,
                    nc=nc,
                ))
            return tuple(outs)
        return _body

    devices = jax.devices()[:NCORES]
    mesh = Mesh(np.asarray(devices), ("core",))

    def _make_exec(reps):
        return jax.jit(
            shard_map(
                _make_body(reps),
                mesh=mesh,
                in_specs=(PartitionSpec("core"),) * (n_params + n_outs),
                out_specs=(PartitionSpec("core"),) * n_outs,
                check_rep=False,
            ),
            donate_argnums=donate,
            keep_unused=True,
        )

    sharded = _make_exec(1)
    _exec_cache = {1: sharded}
    from jax.sharding import NamedSharding
    shard = NamedSharding(mesh, PartitionSpec("core"))

    class Runner:
        def __init__(self):
            self.in_names = in_names
            self.out_names = out_names

        def put_inputs(self, in_maps):
            """Concat per-core inputs and place them on the mesh."""
            import jax as _jax
            concat_in = [
                np.concatenate([np.asarray(m[name]) for m in in_maps], axis=0)
                for name in in_names
            ]
            return [_jax.device_put(a, shard) for a in concat_in]

        def fresh_outs(self):
            import jax as _jax
            return [
                _jax.device_put(np.zeros((NCORES * sh[0], *sh[1:]), dt), shard)
                for sh, dt in zero_shapes
            ]

        def exec_dev(self, dev_in, dev_outs, reps=1):
            """Device step(s). dev_outs is donated; returns new out arrays
            (same shape/sharding — reusable as the next call's dev_outs,
            since the kernel overwrites every output element). reps>1
            chains that many NEFF executions inside one dispatch."""
            if reps not in _exec_cache:
                _exec_cache[reps] = _make_exec(reps)
            return _exec_cache[reps](*dev_in, *dev_outs)

        def run(self, in_maps):
            dev_in = self.put_inputs(in_maps)
            out_arrs = self.exec_dev(dev_in, self.fresh_outs())
            return [
                {
                    name: np.asarray(out_arrs[i]).reshape(
                        NCORES, *out_avals[i].shape)[c]
                    for i, name in enumerate(out_names)
                }
                for c in range(NCORES)
            ]

    _RUNNER = Runner()
    return _RUNNER


def kernel(x, weight, scales):
    runner = _get_runner()
    xf = np.ascontiguousarray(np.asarray(x, dtype=np.float16).reshape(B * S, IN))
    w = np.ascontiguousarray(np.asarray(weight, dtype=np.float16))
    s = np.ascontiguousarray(np.asarray(scales, dtype=np.float16))
    in_maps = [
        {"x": xf[c * M:(c + 1) * M], "w": w, "s": s} for c in range(NCORES)
    ]
    outs = runner.run(in_maps)
    yf = np.concatenate([outs[c]["y"] for c in range(NCORES)], axis=0)
    return yf.reshape(B, S, OUT).astype(np.float16)

